# revision 5
# baseline (speedup 1.0000x reference)
"""Distributed causal RoPE attention for Trainium2 (8 NeuronCores).

Mesh: 2 (batch) x 4 (head-group tensor-parallel).
Core c = b*4 + g handles batch b, heads [4g, 4g+4).

v4 design (vs. v2 phased baseline):
  - Single interleaved schedule: attention block b's scores/PV/projection
    are emitted between the QKV matmuls of later seq chunks, so the
    reduce-scatter stream starts ~90us into the kernel instead of ~290us
    and ScalarE exp overlaps QKV matmuls.
  - QT/KT built with DMA XBAR transposes (SBUF->SBUF, bf16) instead of
    128 PE transpose matmuls + 128 DVE copies.
  - PV computed V-stationary: psum O^T[d, q-block] = sum_kt V_tile.T @
    A^T_tile, streaming 512-wide -- no per-(qt,kt) stationary reloads and
    no PE output transposes.  Softmax denominator comes from DVE/GpSimd
    adds of the A^T tiles + one ones-matmul partition reduce per (h,b);
    normalization is folded into the PSUM->SBUF eviction multiply.
  - exp split across ScalarE and VectorE so PV never waits long on the
    activation stream.
  - Startup DMAs split across both HWDGE queues, first-needed chunks
    first.
"""

import sys

sys.path.insert(0, "/opt/trn_rl_repo")

import numpy as np

import concourse.bass as bass
import concourse.mybir as mybir
import concourse.tile as tile
from concourse.bass_utils import run_bass_kernel_spmd

FP = mybir.dt.float32
BF = mybir.dt.bfloat16
D = 2048  # d_model
S = 2048  # sequence length
B = 2  # batch
NH = 16  # heads
DKV = 128  # head dim
THETA = 10000.0
TP = 4  # head-parallel groups
HPC = NH // TP  # heads per core = 4
HD = HPC * DKV  # head dims per core = 512
NQT = S // 128  # 16 query tiles
NDC = D // 128  # 16 contraction chunks
QB = 4  # q-tiles per attention block
NB = NQT // QB  # 4 blocks
SCALE = 1.0 / float(np.sqrt(DKV))
N_CORES = 8

# reduce-scatter chunks (runs of q-tiles).  Chunks fire as soon as their
# last q-tile is projected; everything but the final chunk completes
# during compute, and the tail is one 4-qt collective.
CHUNK_QTS = [[0, 1, 2], [3, 4, 5, 6], [7, 8, 9, 10, 11], [12, 13, 14, 15]]
RS_CHUNKS = [len(q) for q in CHUNK_QTS]


def _legalize_waits(nc):
    """This walrus build only accepts one embedded sync-wait per TPB
    instruction ("Too many sync wait commands").  Split excess waits of
    compute-engine instructions into preceding engine-local NoOps, each
    carrying a single wait.  DMA (queue-embedded) waits are left alone.
    """
    n_split = 0
    for f in nc.m.functions:
        for bb in f.blocks:
            out = []
            for ins in bb.instructions:
                si = ins.sync_info
                if (
                    si is not None
                    and len(si.on_wait) > 1
                    and ins.engine != mybir.EngineType.Unassigned
                ):
                    waits = {}
                    for w in si.on_wait:
                        key = (w.sync_type, w.id, w.wait_mode)
                        if key not in waits or (
                            w.wait_value is not None
                            and waits[key].wait_value is not None
                            and w.wait_value > waits[key].wait_value
                        ):
                            waits[key] = w
                    waits = list(waits.values())
                    for w in waits[:-1]:
                        nop = mybir.InstNoOp(name=f"{ins.name}-waitsplit-{n_split}")
                        n_split += 1
                        nop.engine = ins.engine
                        nop.sync_info = mybir.SyncInfo(on_wait=[w], on_update=[])
                        out.append(nop)
                    ins.sync_info = mybir.SyncInfo(
                        on_wait=[waits[-1]], on_update=si.on_update
                    )
                out.append(ins)
            bb.instructions = out
    return n_split


def build_nc():
    nc = bass.Bass()

    # Host-pre-transposed, bf16, contiguous DMAs.
    xT = nc.declare_dram_parameter("xT", [NQT, 128, NDC * 128], BF, isOutput=False)
    wq = nc.declare_dram_parameter("wq", [128, NDC * HD], BF, isOutput=False)
    wk = nc.declare_dram_parameter("wk", [128, NDC * HD], BF, isOutput=False)
    wv = nc.declare_dram_parameter("wv", [128, NDC * HD], BF, isOutput=False)
    wo = nc.declare_dram_parameter("wo", [128, HPC * D], BF, isOutput=False)
    cosp = nc.declare_dram_parameter("cosp", [128, NQT * 64], BF, isOutput=False)
    sinp = nc.declare_dram_parameter("sinp", [128, NQT * 64], BF, isOutput=False)
    out = nc.declare_dram_parameter("out", [S // TP, D], BF, isOutput=True)

    with tile.TileContext(nc) as tc:
        with (
            tc.tile_pool(name="dram", bufs=1, space="DRAM") as dram,
            tc.tile_pool(name="const", bufs=1) as constp,
            tc.tile_pool(name="resident", bufs=1) as resp,
            tc.tile_pool(name="wpool", bufs=1) as wpool,
            tc.tile_pool(name="xtp", bufs=3) as xtp,
            tc.tile_pool(name="ropep", bufs=3) as ropep,
            tc.tile_pool(name="tmpp", bufs=2) as tmpp,
            tc.tile_pool(name="atp", bufs=3) as atp,
            tc.tile_pool(name="accp", bufs=2) as accp,
            tc.tile_pool(name="rinvp", bufs=2) as rinvp,
            tc.tile_pool(name="otp", bufs=8) as otp,
            tc.tile_pool(name="outp", bufs=3) as outp,
            tc.tile_pool(name="qkps", bufs=1, space="PSUM") as qkps,
            tc.tile_pool(name="shps", bufs=2, space="PSUM") as shps,
            tc.tile_pool(name="scps", bufs=2, space="PSUM") as scps,
            tc.tile_pool(name="pvps", bufs=2, space="PSUM") as pvps,
        ):
            partials = [
                dram.tile([n * 128, D], BF, name=f"partial{c}", tag=f"partial{c}")
                for c, n in enumerate(RS_CHUNKS)
            ]
            rs_outs = [
                dram.tile([n * 32, D], BF, name=f"rs_out{c}", tag=f"rs_out{c}")
                for c, n in enumerate(RS_CHUNKS)
            ]

            # transposed causal mask for S^T tiles: entry (k, q): keep when
            # q >= k, else -1e10
            cmaskT = constp.tile([128, 128], FP, tag="cmaskT")
            nc.gpsimd.memset(cmaskT[:], 0.0)
            nc.gpsimd.affine_select(
                out=cmaskT[:],
                in_=cmaskT[:],
                compare_op=mybir.AluOpType.is_ge,
                fill=-1e10,
                base=0,
                pattern=[[1, 128]],
                channel_multiplier=-1,
            )
            ones_bf = constp.tile([128, 128], BF, tag="ones")
            nc.vector.memset(ones_bf[:], 1.0)

            # residents: Q^T/K^T [128 (head dim, even|odd basis), HPC*S],
            # V [128 (seq within tile), HPC*NQT*128] (block (h, kt))
            QT = resp.tile([128, HPC * S], BF, tag="QT")
            KT = resp.tile([128, HPC * S], BF, tag="KT")
            V = resp.tile([128, HPC * NQT * 128], BF, tag="V")

            # ---------------- input DMAs ----------------
            # scalar queue: first x tiles + cos/sin (+ later xt tiles and
            # all XBAR transposes).  sync queue: weights (+ partials out).
            cos_sb = wpool.tile([128, NQT * 64], BF, tag="cos")
            sin_sb = wpool.tile([128, NQT * 64], BF, tag="sin")
            xt_pre = {}
            for st in range(2):
                xt_sb = xtp.tile([128, NDC * 128], BF, tag="xt")
                nc.scalar.dma_start(xt_sb[:], xT[st])
                xt_pre[st] = xt_sb
            nc.scalar.dma_start(cos_sb[:, 0 : 4 * 64], cosp[:, 0 : 4 * 64])
            nc.scalar.dma_start(sin_sb[:, 0 : 4 * 64], sinp[:, 0 : 4 * 64])
            nc.scalar.dma_start(cos_sb[:, 4 * 64 :], cosp[:, 4 * 64 :])
            nc.scalar.dma_start(sin_sb[:, 4 * 64 :], sinp[:, 4 * 64 :])

            wq_sb = wpool.tile([128, NDC * HD], BF, tag="wq")
            wk_sb = wpool.tile([128, NDC * HD], BF, tag="wk")
            wv_sb = wpool.tile([128, NDC * HD], BF, tag="wv")
            wo_sb = wpool.tile([128, HPC * D], BF, tag="wo")
            WBND = [0, 2, 6, 11, 16]  # dc boundaries of the weight chunks
            for c in range(4):
                for wsb, wdr in ((wq_sb, wq), (wk_sb, wk), (wv_sb, wv)):
                    nc.sync.dma_start(
                        wsb[:, WBND[c] * HD : WBND[c + 1] * HD],
                        wdr[:, WBND[c] * HD : WBND[c + 1] * HD],
                    )
            nc.sync.dma_start(wo_sb[:], wo[:])

            # ---------------- helpers ----------------
            at_strips = {}  # (b, h) -> strip tile
            acc_tiles = {}  # (b, h) -> denominator accumulator
            rinv_tiles = {}  # (b, h)
            ot_tiles = {}  # (b, h) -> O^T tile [128 d, 512 q]
            done_qts = set()
            fired_chunks = set()

            def emit_scores_h(b, h):
                """scores + exp + denominator adds for (block b, head h)."""
                q0 = b * QB * 128
                strip = atp.tile([128, NQT * 512], BF, tag="at")
                at_strips[(b, h)] = strip
                acc = accp.tile([128, 512], FP, tag="acc")
                acc_tiles[(b, h)] = acc
                nkt = QB * b + QB
                for kt in range(nkt):
                    qlo = max(kt * 128, q0)
                    off = qlo - q0
                    n = 512 - off
                    ps_s = scps.tile([128, 512], FP, tag="sc")
                    nc.tensor.matmul(
                        ps_s[:, :n],
                        KT[:, h * S + kt * 128 : h * S + (kt + 1) * 128],
                        QT[:, h * S + qlo : h * S + q0 + 512],
                        start=True,
                        stop=True,
                    )
                    if kt * 128 >= q0:  # diagonal tile
                        nc.vector.tensor_add(
                            ps_s[:, 0:128], ps_s[:, 0:128], cmaskT[:]
                        )
                    dst = strip[:, kt * 512 + off : (kt + 1) * 512]
                    nc.scalar.activation(
                        dst,
                        ps_s[:, :n],
                        mybir.ActivationFunctionType.Exp,
                        bias=0.0,
                        scale=SCALE,
                    )
                    # denominator accumulation (fp32, SBUF-only engines)
                    if kt == 0:
                        nc.gpsimd.tensor_copy(acc[:], strip[:, 0:512])
                    else:
                        aeng = nc.gpsimd if kt % 2 == 0 else nc.vector
                        aeng.tensor_add(
                            acc[:, off:512], acc[:, off:512], dst
                        )

            def emit_pv_h(b, h):
                """PV + denominator reduce + normalize-evict for (b, h)."""
                strip = at_strips[(b, h)]
                acc = acc_tiles[(b, h)]
                q0 = b * QB * 128
                nkt = QB * b + QB
                # denominator: bf16 cast, ones-matmul partition reduce,
                # reciprocal (replicated across partitions)
                acc_bf = accp.tile([128, 512], BF, tag="accbf")
                nc.vector.tensor_copy(acc_bf[:], acc[:])
                ps_den = scps.tile([128, 512], FP, tag="sc")
                nc.tensor.matmul(
                    ps_den[:], ones_bf[:], acc_bf[:], start=True, stop=True
                )
                rinv = rinvp.tile([128, 512], BF, tag="rinv")
                with nc.allow_low_precision(reason="denominator already bf16-rounded"):
                    nc.vector.reciprocal(rinv[:], ps_den[:])
                rinv_tiles[(b, h)] = rinv
                # PV: V-stationary, A^T streaming, O^T out
                po = pvps.tile([128, 512], FP, tag="pv")
                for kt in range(nkt):
                    off = max(kt * 128, q0) - q0
                    nc.tensor.matmul(
                        po[:, off:512],
                        V[:, (h * NQT + kt) * 128 : (h * NQT + kt) * 128 + 128],
                        strip[:, kt * 512 + off : (kt + 1) * 512],
                        start=(kt == 0),
                        stop=(kt == nkt - 1),
                        skip_group_check=True,
                    )
                ot = otp.tile([128, 512], BF, tag="ot")
                nc.vector.tensor_mul(ot[:], po[:], rinv[:])
                ot_tiles[(b, h)] = ot

            def emit_proj_qt(qt):
                """projection + partial DMA + chunk RS for one q-tile."""
                b = qt // QB
                ql = qt - b * QB
                c = next(i for i, qs in enumerate(CHUNK_QTS) if qt in qs)
                qoff = qt - min(CHUNK_QTS[c])
                for nt in range(D // 512):
                    ps_p = shps.tile([128, 512], FP, tag="sh")
                    for h in range(HPC):
                        nc.tensor.matmul(
                            ps_p[:],
                            ot_tiles[(b, h)][:, ql * 128 : (ql + 1) * 128],
                            wo_sb[:, h * D + nt * 512 : h * D + (nt + 1) * 512],
                            start=(h == 0),
                            stop=(h == HPC - 1),
                        )
                    osb = outp.tile([128, 512], BF, tag="osb")
                    if nt % 2 == 0:
                        nc.scalar.copy(osb[:], ps_p[:])
                    else:
                        nc.vector.tensor_copy(osb[:], ps_p[:])
                    nc.sync.dma_start(
                        partials[c][
                            qoff * 128 : (qoff + 1) * 128,
                            nt * 512 : (nt + 1) * 512,
                        ],
                        osb[:],
                    )
                done_qts.add(qt)
                if c not in fired_chunks and all(
                    q in done_qts for q in CHUNK_QTS[c]
                ):
                    fired_chunks.add(c)
                    nc.gpsimd.collective_compute(
                        "ReduceScatter",
                        mybir.AluOpType.add,
                        replica_groups=[[0, 1, 2, 3], [4, 5, 6, 7]],
                        ins=[partials[c].opt()],
                        outs=[rs_outs[c].opt()],
                    )
                    ooff = min(CHUNK_QTS[c]) * 32
                    nc.gpsimd.dma_start(
                        out[ooff : ooff + RS_CHUNKS[c] * 32, :],
                        rs_outs[c][:, :],
                    )

            # attention payload emitted after each qkv seq-tile st.
            # scores/pv interleaved per head so at most 3 A^T strips are
            # ever live (atp bufs=3).
            PAYLOAD = {
                4: [("s", 0, 0), ("s", 0, 1)],
                5: [("p", 0, 0), ("s", 0, 2), ("p", 0, 1), ("s", 0, 3),
                    ("p", 0, 2), ("p", 0, 3)],
                6: [("j", 0), ("j", 1)],
                7: [("j", 2), ("j", 3)],
                8: [("s", 1, 0), ("s", 1, 1)],
                9: [("p", 1, 0), ("s", 1, 2), ("p", 1, 1)],
                10: [("s", 1, 3), ("p", 1, 2), ("p", 1, 3), ("j", 4)],
                11: [("j", 5), ("j", 6), ("j", 7)],
                12: [("s", 2, 0), ("s", 2, 1)],
                13: [("p", 2, 0), ("s", 2, 2), ("p", 2, 1)],
                14: [("s", 2, 3), ("p", 2, 2), ("p", 2, 3), ("j", 8)],
                15: [("j", 9), ("j", 10), ("j", 11)],
            }

            def attn_payload(st):
                for item in PAYLOAD.get(st, []):
                    if item[0] == "s":
                        emit_scores_h(item[1], item[2])
                    elif item[0] == "p":
                        emit_pv_h(item[1], item[2])
                    else:
                        emit_proj_qt(item[1])

            # ---------------- main interleaved loop ----------------
            for st in range(NQT):
                if st in xt_pre:
                    xt_sb = xt_pre[st]
                else:
                    xt_sb = xtp.tile([128, NDC * 128], BF, tag="xt")
                    nc.scalar.dma_start(xt_sb[:], xT[st])
                ps_q = qkps.tile([128, HD], FP, tag="psq")
                ps_k = qkps.tile([128, HD], FP, tag="psk")
                ps_v = shps.tile([128, HD], FP, tag="sh")
                for dc in range(NDC):
                    lhs = xt_sb[:, dc * 128 : (dc + 1) * 128]
                    for w_sb, ps in ((wq_sb, ps_q), (wk_sb, ps_k), (wv_sb, ps_v)):
                        nc.tensor.matmul(
                            ps[:, :],
                            lhs,
                            w_sb[:, dc * HD : (dc + 1) * HD],
                            start=(dc == 0),
                            stop=(dc == NDC - 1),
                        )
                # V per-head blocks into resident V (cast to bf16)
                for h in range(HPC):
                    nc.vector.tensor_copy(
                        V[:, (h * NQT + st) * 128 : (h * NQT + st) * 128 + 128],
                        ps_v[:, h * 128 : (h + 1) * 128],
                    )
                # RoPE on Q and K (head dims pre-permuted to even|odd
                # halves via host-side W column permutation), then DMA
                # XBAR transpose of each head tile into QT/KT.
                for ps, dst in ((ps_q, QT), (ps_k, KT)):
                    rot = ropep.tile([128, HD], BF, tag="rot")
                    tmp = tmpp.tile([128, HD], FP, tag="tmp")
                    cc = (
                        cos_sb[:, st * 64 : (st + 1) * 64]
                        .rearrange("p (o f) -> p o f", o=1)
                        .broadcast_to((128, HPC, 64))
                    )
                    ss = (
                        sin_sb[:, st * 64 : (st + 1) * 64]
                        .rearrange("p (o f) -> p o f", o=1)
                        .broadcast_to((128, HPC, 64))
                    )
                    psv = ps[:].rearrange("p (h f) -> p h f", h=HPC)
                    rotv = rot[:].rearrange("p (h f) -> p h f", h=HPC)
                    tmpv = tmp[:].rearrange("p (h f) -> p h f", h=HPC)
                    x1 = psv[:, :, 0:64]
                    x2 = psv[:, :, 64:128]
                    t1 = tmpv[:, :, 0:64]
                    t2 = tmpv[:, :, 64:128]
                    nc.vector.tensor_mul(t1, x1, cc)
                    nc.vector.tensor_mul(t2, x2, ss)
                    nc.vector.tensor_sub(rotv[:, :, 0:64], t1, t2)
                    nc.vector.tensor_mul(t1, x1, ss)
                    nc.vector.tensor_mul(t2, x2, cc)
                    nc.vector.tensor_add(rotv[:, :, 64:128], t1, t2)
                    for h in range(HPC):
                        nc.scalar.dma_start(
                            dst[:, h * S + st * 128 : h * S + (st + 1) * 128],
                            rot[:, h * 128 : (h + 1) * 128],
                            transpose=True,
                        )
                attn_payload(st)

            # ---------------- tail: block 3, h-pipelined ----------------
            emit_scores_h(3, 0)
            emit_scores_h(3, 1)
            emit_pv_h(3, 0)
            emit_scores_h(3, 2)
            emit_pv_h(3, 1)
            emit_scores_h(3, 3)
            emit_pv_h(3, 2)
            emit_pv_h(3, 3)
            for qt in (12, 13, 14, 15):
                emit_proj_qt(qt)

    n = _legalize_waits(nc)
    print(f"kernel: split {n} excess sync waits", file=sys.stderr)
    return nc


_NC_CACHE = None
LAST_RESULTS = None


def _ensure_ntff_hook():
    """The agent image's antenv lacks ``axon_hooks``, so the boot-time NTFF
    profile hook registration silently degrades and ``trace=True`` crashes
    on import.  Recreate the module and register the ctypes hook."""
    try:
        from antenv.axon_hooks import get_axon_ntff_profile_hook  # noqa: F401

        return
    except ImportError:
        pass
    import types

    import antenv

    mod = types.ModuleType("antenv.axon_hooks")
    _hook = [None]
    mod.set_axon_ntff_profile_hook = lambda h: _hook.__setitem__(0, h)
    mod.get_axon_ntff_profile_hook = lambda: _hook[0]
    sys.modules["antenv.axon_hooks"] = mod
    antenv.axon_hooks = mod
    if "/root/.axon_site" not in sys.path:
        sys.path.insert(0, "/root/.axon_site")
    from trn_agent_boot.trn_boot import _ntff_profile_via_ctypes

    mod.set_axon_ntff_profile_hook(
        _ntff_profile_via_ctypes("/opt/axon/libaxon_pjrt.so")
    )


def _get_nc():
    global _NC_CACHE
    if _NC_CACHE is None:
        _NC_CACHE = build_nc()
    return _NC_CACHE


def _shard_inputs(x, Wq, Wk, Wv, Wo, token_position):
    import ml_dtypes

    bf16 = ml_dtypes.bfloat16
    x = np.asarray(x, dtype=np.float32)
    Wq = np.asarray(Wq, dtype=np.float32)
    Wk = np.asarray(Wk, dtype=np.float32)
    Wv = np.asarray(Wv, dtype=np.float32)
    Wo = np.asarray(Wo, dtype=np.float32)
    pos = np.asarray(token_position)

    inv_freq = (1.0 / (THETA ** (np.arange(0, DKV, 2, dtype=np.float32) / DKV))).astype(
        np.float32
    )
    ang = pos.astype(np.float32)[:, None] * inv_freq[None, :]
    # host layout [128, NQT*64]: partition p, block st
    cos = np.cos(ang).astype(np.float32).reshape(NQT, 128, 64).transpose(1, 0, 2)
    sin = np.sin(ang).astype(np.float32).reshape(NQT, 128, 64).transpose(1, 0, 2)
    cos = np.ascontiguousarray(cos.reshape(128, NQT * 64)).astype(bf16)
    sin = np.ascontiguousarray(sin.reshape(128, NQT * 64)).astype(bf16)

    # per-head even|odd column permutation for RoPE half-split basis
    perm1 = np.concatenate([np.arange(0, DKV, 2), np.arange(1, DKV, 2)])

    def wlayout(w):  # [D, HD] -> [128, NDC*HD]
        return np.ascontiguousarray(
            w.reshape(NDC, 128, HD).transpose(1, 0, 2).reshape(128, NDC * HD)
        ).astype(bf16)

    in_maps = []
    xT_cache = {}
    for c in range(N_CORES):
        b, g = divmod(c, TP)
        hs = slice(g * HD, (g + 1) * HD)
        permg = np.concatenate([h * DKV + perm1 for h in range(HPC)])
        if b not in xT_cache:
            # [NQT, 128, NDC*128]: element (st, p, c*128+s) = x[b][st*128+s, c*128+p]
            xT_cache[b] = np.ascontiguousarray(
                x[b]
                .T.reshape(NDC, 128, NQT, 128)
                .transpose(2, 1, 0, 3)
                .reshape(NQT, 128, NDC * 128)
            ).astype(bf16)
        wo_g = np.ascontiguousarray(
            Wo[hs, :].reshape(HPC, 128, D).transpose(1, 0, 2).reshape(128, HPC * D)
        ).astype(bf16)
        in_maps.append(
            {
                "xT": xT_cache[b],
                "wq": wlayout(Wq[:, hs][:, permg]),
                "wk": wlayout(Wk[:, hs][:, permg]),
                "wv": wlayout(Wv[:, hs]),
                "wo": wo_g,
                "cosp": cos,
                "sinp": sin,
            }
        )
    return in_maps


def kernel(x, Wq, Wk, Wv, Wo, token_position, trace=False, trace_cores=None):
    global LAST_RESULTS
    if trace:
        _ensure_ntff_hook()
    nc = _get_nc()
    in_maps = _shard_inputs(x, Wq, Wk, Wv, Wo, token_position)
    res = run_bass_kernel_spmd(
        nc,
        in_maps,
        core_ids=list(range(N_CORES)),
        trace=trace,
        trace_cores=trace_cores,
    )
    LAST_RESULTS = res
    out = np.empty((B, S, D), dtype=np.float32)
    for core in range(N_CORES):
        b, g = divmod(core, TP)
        shard = np.asarray(res.results[core]["out"], dtype=np.float32)  # [S//TP, D]
        for qs in CHUNK_QTS:
            lo = min(qs)
            rows = len(qs) * 32
            gstart = lo * 128 + g * rows
            out[b, gstart : gstart + rows, :] = shard[lo * 32 : lo * 32 + rows, :]
    return out


# revision 7
# speedup vs baseline: 1.1116x; 1.1116x over previous
"""Distributed causal RoPE attention for Trainium2 (8 NeuronCores).

Mesh: 2 (batch) x 4 (head-group tensor-parallel).
Core c = b*4 + g handles batch b, heads [4g, 4g+4).

v4 design (vs. v2 phased baseline):
  - Single interleaved schedule: attention block b's scores/PV/projection
    are emitted between the QKV matmuls of later seq chunks, so the
    reduce-scatter stream starts ~90us into the kernel instead of ~290us
    and ScalarE exp overlaps QKV matmuls.
  - QT/KT built with DMA XBAR transposes (SBUF->SBUF, bf16) instead of
    128 PE transpose matmuls + 128 DVE copies.
  - PV computed V-stationary: psum O^T[d, q-block] = sum_kt V_tile.T @
    A^T_tile, streaming 512-wide -- no per-(qt,kt) stationary reloads and
    no PE output transposes.  Softmax denominator comes from DVE/GpSimd
    adds of the A^T tiles + one ones-matmul partition reduce per (h,b);
    normalization is folded into the PSUM->SBUF eviction multiply.
  - exp split across ScalarE and VectorE so PV never waits long on the
    activation stream.
  - Startup DMAs split across both HWDGE queues, first-needed chunks
    first.
"""

import sys

sys.path.insert(0, "/opt/trn_rl_repo")

import numpy as np

import concourse.bass as bass
import concourse.mybir as mybir
import concourse.tile as tile
from concourse.bass_utils import run_bass_kernel_spmd

from concourse.masks import make_identity

FP = mybir.dt.float32
BF = mybir.dt.bfloat16
D = 2048  # d_model
S = 2048  # sequence length
B = 2  # batch
NH = 16  # heads
DKV = 128  # head dim
THETA = 10000.0
TP = 4  # head-parallel groups
HPC = NH // TP  # heads per core = 4
HD = HPC * DKV  # head dims per core = 512
NQT = S // 128  # 16 query tiles
NDC = D // 128  # 16 contraction chunks
QB = 4  # q-tiles per attention block
NB = NQT // QB  # 4 blocks
SCALE = 1.0 / float(np.sqrt(DKV))
N_CORES = 8

# reduce-scatter chunks (runs of q-tiles).  Chunks fire as soon as their
# last q-tile is projected; everything but the final chunk completes
# during compute, and the tail is one 4-qt collective.
CHUNK_QTS = [[0, 1, 2], [3, 4, 5, 6], [7, 8, 9, 10, 11], [12, 13, 14, 15]]
RS_CHUNKS = [len(q) for q in CHUNK_QTS]


def _legalize_waits(nc):
    """This walrus build only accepts one embedded sync-wait per TPB
    instruction ("Too many sync wait commands").  Split excess waits of
    compute-engine instructions into preceding engine-local NoOps, each
    carrying a single wait.  DMA (queue-embedded) waits are left alone.
    """
    n_split = 0
    for f in nc.m.functions:
        for bb in f.blocks:
            out = []
            for ins in bb.instructions:
                si = ins.sync_info
                if (
                    si is not None
                    and len(si.on_wait) > 1
                    and ins.engine != mybir.EngineType.Unassigned
                ):
                    waits = {}
                    for w in si.on_wait:
                        key = (w.sync_type, w.id, w.wait_mode)
                        if key not in waits or (
                            w.wait_value is not None
                            and waits[key].wait_value is not None
                            and w.wait_value > waits[key].wait_value
                        ):
                            waits[key] = w
                    waits = list(waits.values())
                    for w in waits[:-1]:
                        nop = mybir.InstNoOp(name=f"{ins.name}-waitsplit-{n_split}")
                        n_split += 1
                        nop.engine = ins.engine
                        nop.sync_info = mybir.SyncInfo(on_wait=[w], on_update=[])
                        out.append(nop)
                    ins.sync_info = mybir.SyncInfo(
                        on_wait=[waits[-1]], on_update=si.on_update
                    )
                out.append(ins)
            bb.instructions = out
    return n_split


def build_nc():
    nc = bass.Bass()

    # Host-pre-transposed, bf16, contiguous DMAs.
    xT = nc.declare_dram_parameter("xT", [NQT, 128, NDC * 128], BF, isOutput=False)
    wq = nc.declare_dram_parameter("wq", [128, NDC * HD], BF, isOutput=False)
    wk = nc.declare_dram_parameter("wk", [128, NDC * HD], BF, isOutput=False)
    wv = nc.declare_dram_parameter("wv", [128, NDC * HD], BF, isOutput=False)
    wo = nc.declare_dram_parameter("wo", [128, HPC * D], BF, isOutput=False)
    cosp = nc.declare_dram_parameter("cosp", [128, NQT * 64], BF, isOutput=False)
    sinp = nc.declare_dram_parameter("sinp", [128, NQT * 64], BF, isOutput=False)
    out = nc.declare_dram_parameter("out", [S // TP, D], BF, isOutput=True)

    with tile.TileContext(nc) as tc:
        with (
            tc.tile_pool(name="dram", bufs=1, space="DRAM") as dram,
            tc.tile_pool(name="const", bufs=1) as constp,
            tc.tile_pool(name="resident", bufs=1) as resp,
            tc.tile_pool(name="wpool", bufs=1) as wpool,
            tc.tile_pool(name="xtp", bufs=3) as xtp,
            tc.tile_pool(name="ropep", bufs=3) as ropep,
            tc.tile_pool(name="tmpp", bufs=2) as tmpp,
            tc.tile_pool(name="atp", bufs=3) as atp,
            tc.tile_pool(name="accp", bufs=2) as accp,
            tc.tile_pool(name="rinvp", bufs=2) as rinvp,
            tc.tile_pool(name="otp", bufs=8) as otp,
            tc.tile_pool(name="outp", bufs=3) as outp,
            tc.tile_pool(name="qkps", bufs=1, space="PSUM") as qkps,
            tc.tile_pool(name="shps", bufs=2, space="PSUM") as shps,
            tc.tile_pool(name="scps", bufs=2, space="PSUM") as scps,
            tc.tile_pool(name="pvps", bufs=2, space="PSUM") as pvps,
        ):
            partials = [
                dram.tile([n * 128, D], BF, name=f"partial{c}", tag=f"partial{c}")
                for c, n in enumerate(RS_CHUNKS)
            ]
            rs_outs = [
                dram.tile([n * 32, D], BF, name=f"rs_out{c}", tag=f"rs_out{c}")
                for c, n in enumerate(RS_CHUNKS)
            ]

            # transposed causal mask for S^T tiles: entry (k, q): keep when
            # q >= k, else -1e10
            cmaskT = constp.tile([128, 128], FP, tag="cmaskT")
            nc.gpsimd.memset(cmaskT[:], 0.0)
            nc.gpsimd.affine_select(
                out=cmaskT[:],
                in_=cmaskT[:],
                compare_op=mybir.AluOpType.is_ge,
                fill=-1e10,
                base=0,
                pattern=[[1, 128]],
                channel_multiplier=-1,
            )
            ones_bf = constp.tile([128, 128], BF, tag="ones")
            nc.vector.memset(ones_bf[:], 1.0)
            ident_bf = constp.tile([128, 128], BF, tag="ident_bf")
            make_identity(nc, ident_bf[:])

            # residents: Q^T/K^T [128 (head dim, even|odd basis), HPC*S],
            # V [128 (seq within tile), HPC*NQT*128] (block (h, kt))
            QT = resp.tile([128, HPC * S], BF, tag="QT")
            KT = resp.tile([128, HPC * S], BF, tag="KT")
            V = resp.tile([128, HPC * NQT * 128], BF, tag="V")

            # ---------------- input DMAs ----------------
            # scalar queue: first x tiles + cos/sin (+ later xt tiles and
            # all XBAR transposes).  sync queue: weights (+ partials out).
            cos_sb = wpool.tile([128, NQT * 64], BF, tag="cos")
            sin_sb = wpool.tile([128, NQT * 64], BF, tag="sin")
            xt_pre = {}
            for st in range(2):
                xt_sb = xtp.tile([128, NDC * 128], BF, tag="xt")
                nc.scalar.dma_start(xt_sb[:], xT[st])
                xt_pre[st] = xt_sb
            nc.scalar.dma_start(cos_sb[:, 0 : 4 * 64], cosp[:, 0 : 4 * 64])
            nc.scalar.dma_start(sin_sb[:, 0 : 4 * 64], sinp[:, 0 : 4 * 64])
            nc.scalar.dma_start(cos_sb[:, 4 * 64 :], cosp[:, 4 * 64 :])
            nc.scalar.dma_start(sin_sb[:, 4 * 64 :], sinp[:, 4 * 64 :])

            wq_sb = wpool.tile([128, NDC * HD], BF, tag="wq")
            wk_sb = wpool.tile([128, NDC * HD], BF, tag="wk")
            wv_sb = wpool.tile([128, NDC * HD], BF, tag="wv")
            wo_sb = wpool.tile([128, HPC * D], BF, tag="wo")
            WBND = [0, 2, 6, 11, 16]  # dc boundaries of the weight chunks
            for c in range(4):
                for wsb, wdr in ((wq_sb, wq), (wk_sb, wk), (wv_sb, wv)):
                    nc.sync.dma_start(
                        wsb[:, WBND[c] * HD : WBND[c + 1] * HD],
                        wdr[:, WBND[c] * HD : WBND[c + 1] * HD],
                    )
            nc.sync.dma_start(wo_sb[:], wo[:])

            # ---------------- helpers ----------------
            at_strips = {}  # (b, h) -> strip tile
            acc_tiles = {}  # (b, h) -> denominator accumulator
            rinv_tiles = {}  # (b, h)
            ot_tiles = {}  # (b, h) -> O^T tile [128 d, 512 q]
            done_qts = set()
            fired_chunks = set()

            def emit_scores_h(b, h):
                """scores + exp + denominator adds for (block b, head h)."""
                q0 = b * QB * 128
                strip = atp.tile([128, NQT * 512], BF, tag="at")
                at_strips[(b, h)] = strip
                acc = accp.tile([128, 512], FP, tag="acc")
                acc_tiles[(b, h)] = acc
                nkt = QB * b + QB
                for kt in range(nkt):
                    qlo = max(kt * 128, q0)
                    off = qlo - q0
                    n = 512 - off
                    ps_s = scps.tile([128, 512], FP, tag="sc")
                    nc.tensor.matmul(
                        ps_s[:, :n],
                        KT[:, h * S + kt * 128 : h * S + (kt + 1) * 128],
                        QT[:, h * S + qlo : h * S + q0 + 512],
                        start=True,
                        stop=True,
                    )
                    if kt * 128 >= q0:  # diagonal tile
                        nc.vector.tensor_add(
                            ps_s[:, 0:128], ps_s[:, 0:128], cmaskT[:]
                        )
                    dst = strip[:, kt * 512 + off : (kt + 1) * 512]
                    nc.scalar.activation(
                        dst,
                        ps_s[:, :n],
                        mybir.ActivationFunctionType.Exp,
                        bias=0.0,
                        scale=SCALE,
                    )
                    # denominator accumulation (fp32, SBUF-only engines)
                    if kt == 0:
                        nc.gpsimd.tensor_copy(acc[:], strip[:, 0:512])
                    else:
                        nc.gpsimd.tensor_add(
                            acc[:, off:512], acc[:, off:512], dst
                        )

            def emit_pv_h(b, h):
                """PV + denominator reduce + normalize-evict for (b, h)."""
                strip = at_strips[(b, h)]
                acc = acc_tiles[(b, h)]
                q0 = b * QB * 128
                nkt = QB * b + QB
                # denominator: bf16 cast, ones-matmul partition reduce,
                # reciprocal (replicated across partitions)
                acc_bf = accp.tile([128, 512], BF, tag="accbf")
                nc.vector.tensor_copy(acc_bf[:], acc[:])
                ps_den = scps.tile([128, 512], FP, tag="sc")
                nc.tensor.matmul(
                    ps_den[:], ones_bf[:], acc_bf[:], start=True, stop=True
                )
                rinv = rinvp.tile([128, 512], BF, tag="rinv")
                with nc.allow_low_precision(reason="denominator already bf16-rounded"):
                    nc.vector.reciprocal(rinv[:], ps_den[:])
                rinv_tiles[(b, h)] = rinv
                # PV: V-stationary, A^T streaming, O^T out
                po = pvps.tile([128, 512], FP, tag="pv")
                for kt in range(nkt):
                    off = max(kt * 128, q0) - q0
                    nc.tensor.matmul(
                        po[:, off:512],
                        V[:, (h * NQT + kt) * 128 : (h * NQT + kt) * 128 + 128],
                        strip[:, kt * 512 + off : (kt + 1) * 512],
                        start=(kt == 0),
                        stop=(kt == nkt - 1),
                        skip_group_check=True,
                    )
                ot = otp.tile([128, 512], BF, tag="ot")
                nc.vector.tensor_mul(ot[:], po[:], rinv[:])
                ot_tiles[(b, h)] = ot

            def emit_proj_qt(qt):
                """projection + partial DMA + chunk RS for one q-tile."""
                b = qt // QB
                ql = qt - b * QB
                c = next(i for i, qs in enumerate(CHUNK_QTS) if qt in qs)
                qoff = qt - min(CHUNK_QTS[c])
                for nt in range(D // 512):
                    ps_p = shps.tile([128, 512], FP, tag="sh")
                    for h in range(HPC):
                        nc.tensor.matmul(
                            ps_p[:],
                            ot_tiles[(b, h)][:, ql * 128 : (ql + 1) * 128],
                            wo_sb[:, h * D + nt * 512 : h * D + (nt + 1) * 512],
                            start=(h == 0),
                            stop=(h == HPC - 1),
                        )
                    osb = outp.tile([128, 512], BF, tag="osb")
                    if nt % 2 == 0:
                        nc.scalar.copy(osb[:], ps_p[:])
                    else:
                        nc.vector.tensor_copy(osb[:], ps_p[:])
                    nc.sync.dma_start(
                        partials[c][
                            qoff * 128 : (qoff + 1) * 128,
                            nt * 512 : (nt + 1) * 512,
                        ],
                        osb[:],
                    )
                done_qts.add(qt)
                if c not in fired_chunks and all(
                    q in done_qts for q in CHUNK_QTS[c]
                ):
                    fired_chunks.add(c)
                    nc.gpsimd.collective_compute(
                        "ReduceScatter",
                        mybir.AluOpType.add,
                        replica_groups=[[0, 1, 2, 3], [4, 5, 6, 7]],
                        ins=[partials[c].opt()],
                        outs=[rs_outs[c].opt()],
                    )
                    ooff = min(CHUNK_QTS[c]) * 32
                    nc.gpsimd.dma_start(
                        out[ooff : ooff + RS_CHUNKS[c] * 32, :],
                        rs_outs[c][:, :],
                    )

            # attention payload emitted after each qkv seq-tile st.
            # scores/pv interleaved per head so at most 3 A^T strips are
            # ever live (atp bufs=3).
            PAYLOAD = {
                4: [("s", 0, 0), ("s", 0, 1)],
                5: [("p", 0, 0), ("s", 0, 2), ("p", 0, 1), ("s", 0, 3),
                    ("p", 0, 2), ("p", 0, 3)],
                6: [("j", 0), ("j", 1)],
                7: [("j", 2), ("j", 3)],
                8: [("s", 1, 0), ("s", 1, 1)],
                9: [("p", 1, 0), ("s", 1, 2), ("p", 1, 1)],
                10: [("s", 1, 3), ("p", 1, 2), ("p", 1, 3), ("j", 4)],
                11: [("j", 5), ("j", 6), ("j", 7)],
                12: [("s", 2, 0), ("s", 2, 1)],
                13: [("p", 2, 0), ("s", 2, 2), ("p", 2, 1)],
                14: [("s", 2, 3), ("p", 2, 2), ("p", 2, 3), ("j", 8)],
                15: [("j", 9), ("j", 10), ("j", 11)],
            }

            def attn_payload(st):
                for item in PAYLOAD.get(st, []):
                    if item[0] == "s":
                        emit_scores_h(item[1], item[2])
                    elif item[0] == "p":
                        emit_pv_h(item[1], item[2])
                    else:
                        emit_proj_qt(item[1])

            # ---------------- main interleaved loop ----------------
            for st in range(NQT):
                if st in xt_pre:
                    xt_sb = xt_pre[st]
                else:
                    xt_sb = xtp.tile([128, NDC * 128], BF, tag="xt")
                    nc.scalar.dma_start(xt_sb[:], xT[st])
                ps_q = qkps.tile([128, HD], FP, tag="psq")
                ps_k = qkps.tile([128, HD], FP, tag="psk")
                ps_v = shps.tile([128, HD], FP, tag="sh")
                for dc in range(NDC):
                    lhs = xt_sb[:, dc * 128 : (dc + 1) * 128]
                    for w_sb, ps in ((wq_sb, ps_q), (wk_sb, ps_k), (wv_sb, ps_v)):
                        nc.tensor.matmul(
                            ps[:, :],
                            lhs,
                            w_sb[:, dc * HD : (dc + 1) * HD],
                            start=(dc == 0),
                            stop=(dc == NDC - 1),
                        )
                # V per-head blocks into resident V (cast to bf16)
                for h in range(HPC):
                    nc.vector.tensor_copy(
                        V[:, (h * NQT + st) * 128 : (h * NQT + st) * 128 + 128],
                        ps_v[:, h * 128 : (h + 1) * 128],
                    )
                # RoPE on Q and K (head dims pre-permuted to even|odd
                # halves via host-side W column permutation)
                rots = []
                for ps, dst in ((ps_q, QT), (ps_k, KT)):
                    rot = ropep.tile([128, HD], BF, tag="rot")
                    tmp = tmpp.tile([128, HD], FP, tag="tmp")
                    cc = (
                        cos_sb[:, st * 64 : (st + 1) * 64]
                        .rearrange("p (o f) -> p o f", o=1)
                        .broadcast_to((128, HPC, 64))
                    )
                    ss = (
                        sin_sb[:, st * 64 : (st + 1) * 64]
                        .rearrange("p (o f) -> p o f", o=1)
                        .broadcast_to((128, HPC, 64))
                    )
                    psv = ps[:].rearrange("p (h f) -> p h f", h=HPC)
                    rotv = rot[:].rearrange("p (h f) -> p h f", h=HPC)
                    tmpv = tmp[:].rearrange("p (h f) -> p h f", h=HPC)
                    x1 = psv[:, :, 0:64]
                    x2 = psv[:, :, 64:128]
                    t1 = tmpv[:, :, 0:64]
                    t2 = tmpv[:, :, 64:128]
                    nc.vector.tensor_mul(t1, x1, cc)
                    nc.vector.tensor_mul(t2, x2, ss)
                    nc.vector.tensor_sub(rotv[:, :, 0:64], t1, t2)
                    nc.vector.tensor_mul(t1, x1, ss)
                    nc.vector.tensor_mul(t2, x2, cc)
                    nc.vector.tensor_add(rotv[:, :, 64:128], t1, t2)
                    rots.append(rot)
                attn_payload(st)
                # PE transposes of the RoPE'd tiles into QT/KT (after the
                # payload so the PE has work while the DVE RoPE drains)
                for rot, dst in zip(rots, (QT, KT)):
                    for h in range(HPC):
                        tps = shps.tile([128, 512], FP, tag="sh")
                        pt = tps[:, 0:64].bitcast(BF)
                        nc.tensor.transpose(
                            pt, rot[:, h * 128 : (h + 1) * 128], ident_bf[:]
                        )
                        nc.vector.tensor_copy(
                            dst[:, h * S + st * 128 : h * S + (st + 1) * 128],
                            pt,
                        )

            # ---------------- tail: block 3, h-pipelined ----------------
            emit_scores_h(3, 0)
            emit_scores_h(3, 1)
            emit_pv_h(3, 0)
            emit_scores_h(3, 2)
            emit_pv_h(3, 1)
            emit_scores_h(3, 3)
            emit_pv_h(3, 2)
            emit_pv_h(3, 3)
            for qt in (12, 13, 14, 15):
                emit_proj_qt(qt)

    n = _legalize_waits(nc)
    print(f"kernel: split {n} excess sync waits", file=sys.stderr)
    return nc


_NC_CACHE = None
LAST_RESULTS = None


def _ensure_ntff_hook():
    """The agent image's antenv lacks ``axon_hooks``, so the boot-time NTFF
    profile hook registration silently degrades and ``trace=True`` crashes
    on import.  Recreate the module and register the ctypes hook."""
    try:
        from antenv.axon_hooks import get_axon_ntff_profile_hook  # noqa: F401

        return
    except ImportError:
        pass
    import types

    import antenv

    mod = types.ModuleType("antenv.axon_hooks")
    _hook = [None]
    mod.set_axon_ntff_profile_hook = lambda h: _hook.__setitem__(0, h)
    mod.get_axon_ntff_profile_hook = lambda: _hook[0]
    sys.modules["antenv.axon_hooks"] = mod
    antenv.axon_hooks = mod
    if "/root/.axon_site" not in sys.path:
        sys.path.insert(0, "/root/.axon_site")
    from trn_agent_boot.trn_boot import _ntff_profile_via_ctypes

    mod.set_axon_ntff_profile_hook(
        _ntff_profile_via_ctypes("/opt/axon/libaxon_pjrt.so")
    )


def _get_nc():
    global _NC_CACHE
    if _NC_CACHE is None:
        _NC_CACHE = build_nc()
    return _NC_CACHE


def _shard_inputs(x, Wq, Wk, Wv, Wo, token_position):
    import ml_dtypes

    bf16 = ml_dtypes.bfloat16
    x = np.asarray(x, dtype=np.float32)
    Wq = np.asarray(Wq, dtype=np.float32)
    Wk = np.asarray(Wk, dtype=np.float32)
    Wv = np.asarray(Wv, dtype=np.float32)
    Wo = np.asarray(Wo, dtype=np.float32)
    pos = np.asarray(token_position)

    inv_freq = (1.0 / (THETA ** (np.arange(0, DKV, 2, dtype=np.float32) / DKV))).astype(
        np.float32
    )
    ang = pos.astype(np.float32)[:, None] * inv_freq[None, :]
    # host layout [128, NQT*64]: partition p, block st
    cos = np.cos(ang).astype(np.float32).reshape(NQT, 128, 64).transpose(1, 0, 2)
    sin = np.sin(ang).astype(np.float32).reshape(NQT, 128, 64).transpose(1, 0, 2)
    cos = np.ascontiguousarray(cos.reshape(128, NQT * 64)).astype(bf16)
    sin = np.ascontiguousarray(sin.reshape(128, NQT * 64)).astype(bf16)

    # per-head even|odd column permutation for RoPE half-split basis
    perm1 = np.concatenate([np.arange(0, DKV, 2), np.arange(1, DKV, 2)])

    def wlayout(w):  # [D, HD] -> [128, NDC*HD]
        return np.ascontiguousarray(
            w.reshape(NDC, 128, HD).transpose(1, 0, 2).reshape(128, NDC * HD)
        ).astype(bf16)

    in_maps = []
    xT_cache = {}
    for c in range(N_CORES):
        b, g = divmod(c, TP)
        hs = slice(g * HD, (g + 1) * HD)
        permg = np.concatenate([h * DKV + perm1 for h in range(HPC)])
        if b not in xT_cache:
            # [NQT, 128, NDC*128]: element (st, p, c*128+s) = x[b][st*128+s, c*128+p]
            xT_cache[b] = np.ascontiguousarray(
                x[b]
                .T.reshape(NDC, 128, NQT, 128)
                .transpose(2, 1, 0, 3)
                .reshape(NQT, 128, NDC * 128)
            ).astype(bf16)
        wo_g = np.ascontiguousarray(
            Wo[hs, :].reshape(HPC, 128, D).transpose(1, 0, 2).reshape(128, HPC * D)
        ).astype(bf16)
        in_maps.append(
            {
                "xT": xT_cache[b],
                "wq": wlayout(Wq[:, hs][:, permg]),
                "wk": wlayout(Wk[:, hs][:, permg]),
                "wv": wlayout(Wv[:, hs]),
                "wo": wo_g,
                "cosp": cos,
                "sinp": sin,
            }
        )
    return in_maps


def kernel(x, Wq, Wk, Wv, Wo, token_position, trace=False, trace_cores=None):
    global LAST_RESULTS
    if trace:
        _ensure_ntff_hook()
    nc = _get_nc()
    in_maps = _shard_inputs(x, Wq, Wk, Wv, Wo, token_position)
    res = run_bass_kernel_spmd(
        nc,
        in_maps,
        core_ids=list(range(N_CORES)),
        trace=trace,
        trace_cores=trace_cores,
    )
    LAST_RESULTS = res
    out = np.empty((B, S, D), dtype=np.float32)
    for core in range(N_CORES):
        b, g = divmod(core, TP)
        shard = np.asarray(res.results[core]["out"], dtype=np.float32)  # [S//TP, D]
        for qs in CHUNK_QTS:
            lo = min(qs)
            rows = len(qs) * 32
            gstart = lo * 128 + g * rows
            out[b, gstart : gstart + rows, :] = shard[lo * 32 : lo * 32 + rows, :]
    return out


# revision 10
# speedup vs baseline: 1.3864x; 1.2471x over previous
"""Distributed causal RoPE attention for Trainium2 (8 NeuronCores).

Mesh: 2 (batch) x 4 (head-group tensor-parallel).
Core c = b*4 + g handles batch b, heads [4g, 4g+4).

v4 design (vs. v2 phased baseline):
  - Single interleaved schedule: attention block b's scores/PV/projection
    are emitted between the QKV matmuls of later seq chunks, so the
    reduce-scatter stream starts ~90us into the kernel instead of ~290us
    and ScalarE exp overlaps QKV matmuls.
  - QT/KT built with DMA XBAR transposes (SBUF->SBUF, bf16) instead of
    128 PE transpose matmuls + 128 DVE copies.
  - PV computed V-stationary: psum O^T[d, q-block] = sum_kt V_tile.T @
    A^T_tile, streaming 512-wide -- no per-(qt,kt) stationary reloads and
    no PE output transposes.  Softmax denominator comes from DVE/GpSimd
    adds of the A^T tiles + one ones-matmul partition reduce per (h,b);
    normalization is folded into the PSUM->SBUF eviction multiply.
  - exp split across ScalarE and VectorE so PV never waits long on the
    activation stream.
  - Startup DMAs split across both HWDGE queues, first-needed chunks
    first.
"""

import sys

sys.path.insert(0, "/opt/trn_rl_repo")

import numpy as np

import concourse.bass as bass
import concourse.mybir as mybir
import concourse.tile as tile
from concourse.bass_utils import run_bass_kernel_spmd

from concourse.masks import make_identity

FP = mybir.dt.float32
BF = mybir.dt.bfloat16
D = 2048  # d_model
S = 2048  # sequence length
B = 2  # batch
NH = 16  # heads
DKV = 128  # head dim
THETA = 10000.0
TP = 4  # head-parallel groups
HPC = NH // TP  # heads per core = 4
HD = HPC * DKV  # head dims per core = 512
NQT = S // 128  # 16 query tiles
NDC = D // 128  # 16 contraction chunks
QB = 4  # q-tiles per attention block
NB = NQT // QB  # 4 blocks
SCALE = 1.0 / float(np.sqrt(DKV))
N_CORES = 8

# reduce-scatter chunks (runs of q-tiles).  Chunks fire as soon as their
# last q-tile is projected; everything but the final chunk completes
# during compute, and the tail is one 4-qt collective.
CHUNK_QTS = [[0, 1, 2], [3, 4, 5, 6], [7, 8, 9, 10, 11], [12, 13, 14, 15]]
RS_CHUNKS = [len(q) for q in CHUNK_QTS]


def _legalize_waits(nc):
    """This walrus build only accepts one embedded sync-wait per TPB
    instruction ("Too many sync wait commands").  Split excess waits of
    compute-engine instructions into preceding engine-local NoOps, each
    carrying a single wait.  DMA (queue-embedded) waits are left alone.
    """
    n_split = 0
    for f in nc.m.functions:
        for bb in f.blocks:
            out = []
            for ins in bb.instructions:
                si = ins.sync_info
                is_isa = type(ins).__name__ == "InstISA"
                if (
                    si is not None
                    and len(si.on_wait) > (0 if is_isa else 1)
                    and ins.engine != mybir.EngineType.Unassigned
                ):
                    waits = {}
                    for w in si.on_wait:
                        key = (w.sync_type, w.id, w.wait_mode)
                        if key not in waits or (
                            w.wait_value is not None
                            and waits[key].wait_value is not None
                            and w.wait_value > waits[key].wait_value
                        ):
                            waits[key] = w
                    waits = list(waits.values())
                    keep = [] if is_isa else [waits[-1]]
                    for w in (waits if is_isa else waits[:-1]):
                        nop = mybir.InstNoOp(name=f"{ins.name}-waitsplit-{n_split}")
                        n_split += 1
                        nop.engine = ins.engine
                        nop.sync_info = mybir.SyncInfo(on_wait=[w], on_update=[])
                        out.append(nop)
                    ins.sync_info = mybir.SyncInfo(
                        on_wait=keep, on_update=si.on_update
                    )
                out.append(ins)
            bb.instructions = out
    return n_split


def build_nc():
    nc = bass.Bass()

    # Host-pre-transposed, bf16, contiguous DMAs.
    xT = nc.declare_dram_parameter("xT", [NQT, 128, NDC * 128], BF, isOutput=False)
    wq = nc.declare_dram_parameter("wq", [128, NDC * HD], BF, isOutput=False)
    wk = nc.declare_dram_parameter("wk", [128, NDC * HD], BF, isOutput=False)
    wv = nc.declare_dram_parameter("wv", [128, NDC * HD], BF, isOutput=False)
    wo = nc.declare_dram_parameter("wo", [128, HPC * D], BF, isOutput=False)
    cosp = nc.declare_dram_parameter("cosp", [128, NQT * 64], BF, isOutput=False)
    sinp = nc.declare_dram_parameter("sinp", [128, NQT * 64], BF, isOutput=False)
    out = nc.declare_dram_parameter("out", [S // TP, D], BF, isOutput=True)

    with tile.TileContext(nc) as tc:
        with (
            tc.tile_pool(name="dram", bufs=1, space="DRAM") as dram,
            tc.tile_pool(name="const", bufs=1) as constp,
            tc.tile_pool(name="resident", bufs=1) as resp,
            tc.tile_pool(name="wpool", bufs=1) as wpool,
            tc.tile_pool(name="xtp", bufs=3) as xtp,
            tc.tile_pool(name="ropep", bufs=3) as ropep,
            tc.tile_pool(name="tmpp", bufs=2) as tmpp,
            tc.tile_pool(name="atp", bufs=3) as atp,
            tc.tile_pool(name="rinvp", bufs=2) as rinvp,
            tc.tile_pool(name="otp", bufs=8) as otp,
            tc.tile_pool(name="outp", bufs=3) as outp,
            tc.tile_pool(name="qkps", bufs=1, space="PSUM") as qkps,
            tc.tile_pool(name="shps", bufs=2, space="PSUM") as shps,
            tc.tile_pool(name="scps", bufs=2, space="PSUM") as scps,
            tc.tile_pool(name="pvps", bufs=2, space="PSUM") as pvps,
        ):
            partials = [
                dram.tile([n * 128, D], BF, name=f"partial{c}", tag=f"partial{c}")
                for c, n in enumerate(RS_CHUNKS)
            ]
            rs_outs = [
                dram.tile([n * 32, D], BF, name=f"rs_out{c}", tag=f"rs_out{c}")
                for c, n in enumerate(RS_CHUNKS)
            ]

            # transposed causal mask for S^T tiles: entry (k, q): keep when
            # q >= k, else -1e10
            cmaskT = constp.tile([128, 128], FP, tag="cmaskT")
            nc.gpsimd.memset(cmaskT[:], 0.0)
            nc.gpsimd.affine_select(
                out=cmaskT[:],
                in_=cmaskT[:],
                compare_op=mybir.AluOpType.is_ge,
                fill=-1e10,
                base=0,
                pattern=[[1, 128]],
                channel_multiplier=-1,
            )
            ones_bf = constp.tile([128, 128], BF, tag="ones")
            nc.vector.memset(ones_bf[:], 1.0)
            ident_bf = constp.tile([128, 128], BF, tag="ident_bf")
            make_identity(nc, ident_bf[:])

            # residents: Q^T/K^T [128 (head dim, even|odd basis), HPC*S],
            # V [128 (seq within tile), HPC*NQT*128] (block (h, kt))
            QT = resp.tile([128, HPC * S], BF, tag="QT")
            KT = resp.tile([128, HPC * S], BF, tag="KT")
            V = resp.tile([128, HPC * NQT * 128], BF, tag="V")

            # ---------------- input DMAs ----------------
            # scalar queue: first x tiles + cos/sin (+ later xt tiles and
            # all XBAR transposes).  sync queue: weights (+ partials out).
            cos_sb = wpool.tile([128, NQT * 64], BF, tag="cos")
            sin_sb = wpool.tile([128, NQT * 64], BF, tag="sin")
            xt_pre = {}
            for st in range(2):
                xt_sb = xtp.tile([128, NDC * 128], BF, tag="xt")
                nc.scalar.dma_start(xt_sb[:], xT[st])
                xt_pre[st] = xt_sb
            nc.scalar.dma_start(cos_sb[:, 0 : 4 * 64], cosp[:, 0 : 4 * 64])
            nc.scalar.dma_start(sin_sb[:, 0 : 4 * 64], sinp[:, 0 : 4 * 64])
            nc.scalar.dma_start(cos_sb[:, 4 * 64 :], cosp[:, 4 * 64 :])
            nc.scalar.dma_start(sin_sb[:, 4 * 64 :], sinp[:, 4 * 64 :])

            wq_sb = wpool.tile([128, NDC * HD], BF, tag="wq")
            wk_sb = wpool.tile([128, NDC * HD], BF, tag="wk")
            wv_sb = wpool.tile([128, NDC * HD], BF, tag="wv")
            wo_sb = wpool.tile([128, HPC * D], BF, tag="wo")
            WBND = [0, 2, 6, 11, 16]  # dc boundaries of the weight chunks
            for c in range(4):
                for wsb, wdr in ((wq_sb, wq), (wk_sb, wk), (wv_sb, wv)):
                    nc.sync.dma_start(
                        wsb[:, WBND[c] * HD : WBND[c + 1] * HD],
                        wdr[:, WBND[c] * HD : WBND[c + 1] * HD],
                    )
            nc.sync.dma_start(wo_sb[:], wo[:])

            # ---------------- helpers ----------------
            at_strips = {}  # (b, h) -> strip tile
            rinv_tiles = {}  # (b, h)
            ot_tiles = {}  # (b, h) -> O^T tile [128 d, 512 q]
            done_qts = set()
            fired_chunks = set()

            def emit_scores_h(b, h):
                """scores + exp + denominator adds for (block b, head h)."""
                q0 = b * QB * 128
                strip = atp.tile([128, NQT * 512], BF, tag="at")
                at_strips[(b, h)] = strip
                nkt = QB * b + QB
                for kt in range(nkt):
                    qlo = max(kt * 128, q0)
                    off = qlo - q0
                    n = 512 - off
                    ps_s = scps.tile([128, 512], FP, tag="sc")
                    nc.tensor.matmul(
                        ps_s[:, :n],
                        KT[:, h * S + kt * 128 : h * S + (kt + 1) * 128],
                        QT[:, h * S + qlo : h * S + q0 + 512],
                        start=True,
                        stop=True,
                    )
                    if kt * 128 >= q0:  # diagonal tile
                        nc.vector.tensor_add(
                            ps_s[:, 0:128], ps_s[:, 0:128], cmaskT[:]
                        )
                    dst = strip[:, kt * 512 + off : (kt + 1) * 512]
                    nc.scalar.activation(
                        dst,
                        ps_s[:, :n],
                        mybir.ActivationFunctionType.Exp,
                        bias=0.0,
                        scale=SCALE,
                    )


            def emit_pv_h(b, h):
                """PV + denominator + normalize-evict for (b, h).

                PV (V-stationary, A^T streaming, O^T out) interleaved with
                ones-matmuls accumulating the softmax denominator --
                replicated across partitions -- in a second psum bank.
                """
                strip = at_strips[(b, h)]
                q0 = b * QB * 128
                nkt = QB * b + QB
                po = pvps.tile([128, 512], FP, tag="pv")
                ps_den = pvps.tile([128, 512], FP, tag="pv")
                for kt in range(nkt):
                    off = max(kt * 128, q0) - q0
                    sl = strip[:, kt * 512 + off : (kt + 1) * 512]
                    nc.tensor.matmul(
                        po[:, off:512],
                        V[:, (h * NQT + kt) * 128 : (h * NQT + kt) * 128 + 128],
                        sl,
                        start=(kt == 0),
                        stop=(kt == nkt - 1),
                        skip_group_check=True,
                    )
                    nc.tensor.matmul(
                        ps_den[:, off:512],
                        ones_bf[:],
                        sl,
                        start=(kt == 0),
                        stop=(kt == nkt - 1),
                        skip_group_check=True,
                    )
                rinv = rinvp.tile([128, 512], BF, tag="rinv")
                with nc.allow_low_precision(reason="denominator rinv bf16"):
                    nc.vector.reciprocal(rinv[:], ps_den[:])
                rinv_tiles[(b, h)] = rinv
                ot = otp.tile([128, 512], BF, tag="ot")
                nc.vector.tensor_mul(ot[:], po[:], rinv[:])
                ot_tiles[(b, h)] = ot

            def emit_proj_qt(qt):
                """projection + partial DMA + chunk RS for one q-tile."""
                b = qt // QB
                ql = qt - b * QB
                c = next(i for i, qs in enumerate(CHUNK_QTS) if qt in qs)
                qoff = qt - min(CHUNK_QTS[c])
                for nt in range(D // 512):
                    ps_p = shps.tile([128, 512], FP, tag="sh")
                    for h in range(HPC):
                        nc.tensor.matmul(
                            ps_p[:],
                            ot_tiles[(b, h)][:, ql * 128 : (ql + 1) * 128],
                            wo_sb[:, h * D + nt * 512 : h * D + (nt + 1) * 512],
                            start=(h == 0),
                            stop=(h == HPC - 1),
                        )
                    osb = outp.tile([128, 512], BF, tag="osb")
                    nc.scalar.copy(osb[:], ps_p[:])
                    nc.sync.dma_start(
                        partials[c][
                            qoff * 128 : (qoff + 1) * 128,
                            nt * 512 : (nt + 1) * 512,
                        ],
                        osb[:],
                    )
                done_qts.add(qt)
                if c not in fired_chunks and all(
                    q in done_qts for q in CHUNK_QTS[c]
                ):
                    fired_chunks.add(c)
                    nc.gpsimd.collective_compute(
                        "ReduceScatter",
                        mybir.AluOpType.add,
                        replica_groups=[[0, 1, 2, 3], [4, 5, 6, 7]],
                        ins=[partials[c].opt()],
                        outs=[rs_outs[c].opt()],
                    )
                    ooff = min(CHUNK_QTS[c]) * 32
                    nc.gpsimd.dma_start(
                        out[ooff : ooff + RS_CHUNKS[c] * 32, :],
                        rs_outs[c][:, :],
                    )

            # attention payload emitted after each qkv seq-tile st.
            # scores/pv interleaved per head so at most 3 A^T strips are
            # ever live (atp bufs=3).
            PAYLOAD = {
                4: [("s", 0, 0), ("s", 0, 1)],
                5: [("p", 0, 0), ("s", 0, 2), ("p", 0, 1), ("s", 0, 3),
                    ("p", 0, 2), ("p", 0, 3)],
                6: [("j", 0), ("j", 1)],
                7: [("j", 2), ("j", 3)],
                8: [("s", 1, 0), ("s", 1, 1)],
                9: [("p", 1, 0), ("s", 1, 2), ("p", 1, 1)],
                10: [("s", 1, 3), ("p", 1, 2), ("p", 1, 3), ("j", 4)],
                11: [("j", 5), ("j", 6), ("j", 7)],
                12: [("s", 2, 0), ("s", 2, 1)],
                13: [("p", 2, 0), ("s", 2, 2), ("p", 2, 1)],
                14: [("s", 2, 3), ("p", 2, 2), ("p", 2, 3), ("j", 8)],
                15: [("j", 9), ("j", 10), ("j", 11)],
            }

            def attn_payload(st):
                for item in PAYLOAD.get(st, []):
                    if item[0] == "s":
                        emit_scores_h(item[1], item[2])
                    elif item[0] == "p":
                        emit_pv_h(item[1], item[2])
                    else:
                        emit_proj_qt(item[1])

            # ---------------- main interleaved loop ----------------
            for st in range(NQT):
                if st in xt_pre:
                    xt_sb = xt_pre[st]
                else:
                    xt_sb = xtp.tile([128, NDC * 128], BF, tag="xt")
                    nc.scalar.dma_start(xt_sb[:], xT[st])
                ps_q = qkps.tile([128, HD], FP, tag="psq")
                ps_k = qkps.tile([128, HD], FP, tag="psk")
                ps_v = shps.tile([128, HD], FP, tag="sh")
                for dc in range(NDC):
                    lhs = xt_sb[:, dc * 128 : (dc + 1) * 128]
                    for w_sb, ps in ((wq_sb, ps_q), (wk_sb, ps_k), (wv_sb, ps_v)):
                        nc.tensor.matmul(
                            ps[:, :],
                            lhs,
                            w_sb[:, dc * HD : (dc + 1) * HD],
                            start=(dc == 0),
                            stop=(dc == NDC - 1),
                        )
                # V per-head blocks into resident V (cast to bf16)
                for h in range(HPC):
                    nc.scalar.copy(
                        V[:, (h * NQT + st) * 128 : (h * NQT + st) * 128 + 128],
                        ps_v[:, h * 128 : (h + 1) * 128],
                    )
                # RoPE on Q and K (head dims pre-permuted to even|odd
                # halves via host-side W column permutation)
                rots = []
                for ps, dst in ((ps_q, QT), (ps_k, KT)):
                    rot = ropep.tile([128, HD], BF, tag="rot")
                    tmp = tmpp.tile([128, HD], FP, tag="tmp")
                    cc = (
                        cos_sb[:, st * 64 : (st + 1) * 64]
                        .rearrange("p (o f) -> p o f", o=1)
                        .broadcast_to((128, HPC, 64))
                    )
                    ss = (
                        sin_sb[:, st * 64 : (st + 1) * 64]
                        .rearrange("p (o f) -> p o f", o=1)
                        .broadcast_to((128, HPC, 64))
                    )
                    psv = ps[:].rearrange("p (h f) -> p h f", h=HPC)
                    rotv = rot[:].rearrange("p (h f) -> p h f", h=HPC)
                    tmpv = tmp[:].rearrange("p (h f) -> p h f", h=HPC)
                    x1 = psv[:, :, 0:64]
                    x2 = psv[:, :, 64:128]
                    t1 = tmpv[:, :, 0:64]
                    t2 = tmpv[:, :, 64:128]
                    nc.vector.tensor_mul(t1, x1, cc)
                    nc.vector.tensor_mul(t2, x2, ss)
                    nc.vector.tensor_sub(rotv[:, :, 0:64], t1, t2)
                    nc.vector.tensor_mul(t1, x1, ss)
                    nc.vector.tensor_mul(t2, x2, cc)
                    nc.vector.tensor_add(rotv[:, :, 64:128], t1, t2)
                    rots.append(rot)
                attn_payload(st)
                # PE transposes of the RoPE'd tiles into QT/KT (after the
                # payload so the PE has work while the DVE RoPE drains)
                for rot, dst in zip(rots, (QT, KT)):
                    for h in range(HPC):
                        tps = shps.tile([128, 512], FP, tag="sh")
                        pt = tps[:, 0:64].bitcast(BF)
                        nc.tensor.transpose(
                            pt, rot[:, h * 128 : (h + 1) * 128], ident_bf[:]
                        )
                        nc.vector.tensor_copy(
                            dst[:, h * S + st * 128 : h * S + (st + 1) * 128],
                            pt,
                        )

            # ---------------- tail: block 3, h-pipelined ----------------
            emit_scores_h(3, 0)
            emit_scores_h(3, 1)
            emit_pv_h(3, 0)
            emit_scores_h(3, 2)
            emit_pv_h(3, 1)
            emit_scores_h(3, 3)
            emit_pv_h(3, 2)
            emit_pv_h(3, 3)
            for qt in (12, 13, 14, 15):
                emit_proj_qt(qt)

    n = _legalize_waits(nc)
    print(f"kernel: split {n} excess sync waits", file=sys.stderr)
    return nc


_NC_CACHE = None
LAST_RESULTS = None


def _ensure_ntff_hook():
    """The agent image's antenv lacks ``axon_hooks``, so the boot-time NTFF
    profile hook registration silently degrades and ``trace=True`` crashes
    on import.  Recreate the module and register the ctypes hook."""
    try:
        from antenv.axon_hooks import get_axon_ntff_profile_hook  # noqa: F401

        return
    except ImportError:
        pass
    import types

    import antenv

    mod = types.ModuleType("antenv.axon_hooks")
    _hook = [None]
    mod.set_axon_ntff_profile_hook = lambda h: _hook.__setitem__(0, h)
    mod.get_axon_ntff_profile_hook = lambda: _hook[0]
    sys.modules["antenv.axon_hooks"] = mod
    antenv.axon_hooks = mod
    if "/root/.axon_site" not in sys.path:
        sys.path.insert(0, "/root/.axon_site")
    from trn_agent_boot.trn_boot import _ntff_profile_via_ctypes

    mod.set_axon_ntff_profile_hook(
        _ntff_profile_via_ctypes("/opt/axon/libaxon_pjrt.so")
    )


def _get_nc():
    global _NC_CACHE
    if _NC_CACHE is None:
        _NC_CACHE = build_nc()
    return _NC_CACHE


def _shard_inputs(x, Wq, Wk, Wv, Wo, token_position):
    import ml_dtypes

    bf16 = ml_dtypes.bfloat16
    x = np.asarray(x, dtype=np.float32)
    Wq = np.asarray(Wq, dtype=np.float32)
    Wk = np.asarray(Wk, dtype=np.float32)
    Wv = np.asarray(Wv, dtype=np.float32)
    Wo = np.asarray(Wo, dtype=np.float32)
    pos = np.asarray(token_position)

    inv_freq = (1.0 / (THETA ** (np.arange(0, DKV, 2, dtype=np.float32) / DKV))).astype(
        np.float32
    )
    ang = pos.astype(np.float32)[:, None] * inv_freq[None, :]
    # host layout [128, NQT*64]: partition p, block st
    cos = np.cos(ang).astype(np.float32).reshape(NQT, 128, 64).transpose(1, 0, 2)
    sin = np.sin(ang).astype(np.float32).reshape(NQT, 128, 64).transpose(1, 0, 2)
    cos = np.ascontiguousarray(cos.reshape(128, NQT * 64)).astype(bf16)
    sin = np.ascontiguousarray(sin.reshape(128, NQT * 64)).astype(bf16)

    # per-head even|odd column permutation for RoPE half-split basis
    perm1 = np.concatenate([np.arange(0, DKV, 2), np.arange(1, DKV, 2)])

    def wlayout(w):  # [D, HD] -> [128, NDC*HD]
        return np.ascontiguousarray(
            w.reshape(NDC, 128, HD).transpose(1, 0, 2).reshape(128, NDC * HD)
        ).astype(bf16)

    in_maps = []
    xT_cache = {}
    for c in range(N_CORES):
        b, g = divmod(c, TP)
        hs = slice(g * HD, (g + 1) * HD)
        permg = np.concatenate([h * DKV + perm1 for h in range(HPC)])
        if b not in xT_cache:
            # [NQT, 128, NDC*128]: element (st, p, c*128+s) = x[b][st*128+s, c*128+p]
            xT_cache[b] = np.ascontiguousarray(
                x[b]
                .T.reshape(NDC, 128, NQT, 128)
                .transpose(2, 1, 0, 3)
                .reshape(NQT, 128, NDC * 128)
            ).astype(bf16)
        wo_g = np.ascontiguousarray(
            Wo[hs, :].reshape(HPC, 128, D).transpose(1, 0, 2).reshape(128, HPC * D)
        ).astype(bf16)
        in_maps.append(
            {
                "xT": xT_cache[b],
                "wq": wlayout(Wq[:, hs][:, permg]),
                "wk": wlayout(Wk[:, hs][:, permg]),
                "wv": wlayout(Wv[:, hs]),
                "wo": wo_g,
                "cosp": cos,
                "sinp": sin,
            }
        )
    return in_maps


def kernel(x, Wq, Wk, Wv, Wo, token_position, trace=False, trace_cores=None):
    global LAST_RESULTS
    if trace:
        _ensure_ntff_hook()
    nc = _get_nc()
    in_maps = _shard_inputs(x, Wq, Wk, Wv, Wo, token_position)
    res = run_bass_kernel_spmd(
        nc,
        in_maps,
        core_ids=list(range(N_CORES)),
        trace=trace,
        trace_cores=trace_cores,
    )
    LAST_RESULTS = res
    out = np.empty((B, S, D), dtype=np.float32)
    for core in range(N_CORES):
        b, g = divmod(core, TP)
        shard = np.asarray(res.results[core]["out"], dtype=np.float32)  # [S//TP, D]
        for qs in CHUNK_QTS:
            lo = min(qs)
            rows = len(qs) * 32
            gstart = lo * 128 + g * rows
            out[b, gstart : gstart + rows, :] = shard[lo * 32 : lo * 32 + rows, :]
    return out
